# revision 24
# baseline (speedup 1.0000x reference)
"""Trainium2 Bass kernel for a dense transformer block (LN-attn-LN-MLP).

Sharding: 8 cores = (4 batches) x (2 query-halves). Each core computes k/v for
its batch's full 2048 tokens (duplicated within the pair; avoids collectives)
and queries/MLP for its own 1024 tokens. All activations are kept
feature-major ([C, T]); the host passes x pre-transposed (bf16 only) and
un-transposes the output. The device returns x2 + mlp where x2 = bf16(x) +
attn_proj; the host adds back the f32-vs-bf16 residue of x.

Precision plan (validated in sim_precision.py, rel-err ~1.45e-2):
  qkv + v GEMMs: fp8e4m3 DoubleRow;  scores: bf16
  exp: s=0 half on Act (exact), s=1 half on DVE via the Schraudolph
    int16-in-bf16 bit trick (bits = round(184.665*s + 16250.5) == exp(s)
    within +-3%; softmax normalization cancels the systematic part)
  attn*V: bf16, 65-row v_aug with ones column for the denominators
  softmax reciprocal: exp(-ln(z)) on Act (same activation table set)
  proj: fp8e4m3 DR (val6 quantized during the fused normalize drain)
  fc1: bf16 (was the dominant error term; buys budget for fp8 fc2)
  fc2: fp8e4m3 DR (gelu writes hid as f8 directly)
  LN rsqrt: exp(-0.5*ln(var+eps)) on Act
The activation-table map is patched so exp/ln/square all resolve to the
natural_log_exp_and_others set (one load instead of per-function thrash).

Score matmuls for the two heads of a pair are issued adjacently with K=64
on partition rows 0-63 / 64-127 so the PE runs them concurrently (row-group
packing); both exps are issued before either head's attn*V so the strict-
FIFO PE queue never stalls on the Act/DVE exp.
"""
import sys
sys.path.insert(0, "/opt/trn_rl_repo")

import math

import numpy as np
import ml_dtypes

import concourse.bass as bass
import concourse.tile as tile
from concourse import bacc
from concourse import mybir
from concourse.bass_utils import run_bass_kernel_spmd
from concourse.hw_specs import get_activation_tables

F32 = mybir.dt.float32
BF16 = mybir.dt.bfloat16
F8 = mybir.dt.float8e4
I16 = mybir.dt.int16
AF = mybir.ActivationFunctionType
OP = mybir.AluOpType
DR = mybir.MatmulPerfMode.DoubleRow

P = 128
C = 768            # embed dim
CO = 6             # C / 128 chunks
H = 12             # heads
D = 64             # head dim
HID = 3072
HF = 24            # HID / 128 chunks
TK = 2048          # tokens per batch (keys/values)
TQ = 1024          # query tokens per core
NKT = TK // P      # 16 key tiles
NTC = TK // 512    # 4 token chunks (LN1)
NQC = TQ // 512    # 2 query chunks
LN_EPS = 1e-6
NPAIR = 6          # head pairs
VROW = 12 * 65     # v_aug row: 12 heads x (64 + ones col)

# Schraudolph exp constants for the bf16 bit pattern: bits = A*s + B
EXP_A = 128.0 / math.log(2.0)
EXP_B = 16256.0 - 5.51          # 127*128 with sawtooth-centering correction
S_V6 = 16.0                      # f8 scale for normalized attn values
LOG_SV6 = math.log(S_V6)
S_VT = 64.0                      # f8 scale for even-head v values
VP2 = 400                        # padded row for split v_aug (6*65 -> 400, %16)


def _patch_act_tables(arch):
    """Make exp/ln/square resolve to natural_log_exp_and_others so the
    LN-rsqrt (ln+exp), softmax exp and softmax reciprocal (ln+exp) share
    one activation table set instead of thrashing loads."""
    tabs = get_activation_tables(arch)
    comb = "natural_log_exp_and_others"
    for name, fset in tabs.items():
        if name == comb:
            continue
        if name.startswith("gelu"):
            continue
        fset.discard(AF.Exp)
        fset.discard(AF.Ln)
        fset.discard(AF.Square)
    return tabs


def _build_nc(flags, inv):
    """Build the single-core Bass program (identical for all 8 cores)."""
    nc = bacc.Bacc()
    _patch_act_tables(nc.m.arch)

    xTb_d = nc.declare_dram_parameter("xTb", [C, TK], BF16, isOutput=False)
    wqk_d = nc.declare_dram_parameter("wqk", [12, P, CO, P], F8, isOutput=False)
    wv_d = nc.declare_dram_parameter("wv", [P, CO, C], F8, isOutput=False)
    pjw_d = nc.declare_dram_parameter("pjw", [CO, P, CO, P], F8, isOutput=False)
    f1w_d = nc.declare_dram_parameter("f1w", [HF, P, CO, P], BF16, isOutput=False)
    f2w_d = nc.declare_dram_parameter("f2w", [CO, P, HF, P], F8, isOutput=False)
    texp_d = nc.declare_dram_parameter("texp", [P, NPAIR], F32, isOutput=False)
    if flags["qkb"]:
        qkb_d = nc.declare_dram_parameter("qkb", [P, 12], F32, isOutput=False)
    if flags["pjb"]:
        pjb_d = nc.declare_dram_parameter("pjb", [P, CO], F32, isOutput=False)
    if flags["f1b"]:
        f1b_d = nc.declare_dram_parameter("f1b", [P, HF], F32, isOutput=False)
    if flags["f2b"]:
        f2b_d = nc.declare_dram_parameter("f2b", [P, CO], F32, isOutput=False)
    if flags["ln1_aff"]:
        l1g_d = nc.declare_dram_parameter("l1g", [P, CO], F32, isOutput=False)
        l1b_d = nc.declare_dram_parameter("l1b", [P, CO], F32, isOutput=False)
    if flags["ln2_aff"]:
        l2g_d = nc.declare_dram_parameter("l2g", [P, CO], F32, isOutput=False)
        l2b_d = nc.declare_dram_parameter("l2b", [P, CO], F32, isOutput=False)
    if flags["vbias"]:
        vbb_d = nc.declare_dram_parameter("vbb", [1, C], F32, isOutput=False)
    if flags["mask"]:
        mq_d = nc.declare_dram_parameter("mq", [1, TQ], F32, isOutput=False)
    out_d = nc.declare_dram_parameter("outT", [C, TQ], BF16, isOutput=True)

    xTb3 = xTb_d.rearrange("(co ci) t -> ci co t", ci=P)

    from contextlib import ExitStack
    with tile.TileContext(nc) as tc, ExitStack() as ctx:
        pool = lambda name, bufs, **kw: ctx.enter_context(
            tc.tile_pool(name=name, bufs=bufs, **kw))
        px = pool("px", 3)          # x6 chunks [128,6,512] bf16 (0,1 resident)
        psq = pool("psq", 2)        # LN sq [128,512] bf16 (1KB)
        ph1 = pool("ph1", 4)        # h1t [128,6,512] f8 (4 t-chunks resident)
        ph2 = pool("ph2", 2)        # h2t [128,6,512] bf16
        phid = pool("phid", 2)      # hid [128,24,512] f8 per qc
        pw = pool("pw", 3)          # [128,6,128] weight tiles
        pwb = pool("pwb", 3)        # wv / fc2 big weight tiles
        pkr = pool("pkr", 6)        # resident kT [128,2048] bf16
        pqr = pool("pqr", 6)        # resident qT [128,1024] bf16
        pvf = pool("pvf", 8)        # even-head v_aug pairs [128,2,400] f8
        pvb = pool("pvb", 8)        # odd-head v_aug pairs [128,2,400] bf16
        pvl = pool("pvl", 2)        # val6 f8 [128,6,512] per qc
        ppd = pool("ppd", 6)        # unnormalized head sums [64,512] bf16
        px2 = pool("px2", 2)        # x2 [128,6,512] bf16 per qc
        pat = pool("pat", 2)        # attn exp out f8 [128,1024]
        patb = pool("patb", 2)      # attn exp out bf16 [128,1024]
        pxy = pool("pxy", 2)        # out [128,512] f32
        pst = pool("pst", 4)        # stat bcast [128,512]
        pu = pool("pu", 2)          # LN centered [128,512]
        prow = pool("prow", 5)      # [1..2,512] rows
        prb = pool("prb", 4)        # bcast ln rows [64,512] f32
        prc = pool("prc", 4)        # rb [64,512] bf16
        pone = pool("pone", 1)
        psA = pool("psA", 3, space="PSUM")  # [128,1024] 2-bank slots
        psB = pool("psB", 2, space="PSUM")  # 1-bank slots (pv / LN stats)
        if True:
            ones_b = pone.tile([P, 1], BF16, tag="ones_b")
            nc.vector.memset(ones_b, 1.0)
            eps_sb = pone.tile([P, 1], F32, tag="eps")
            nc.vector.memset(eps_sb, LN_EPS)
            lsv6 = pone.tile([P, 1], F32, tag="lsv6")
            nc.vector.memset(lsv6, LOG_SV6)
            rbias0 = pone.tile([D, 1], F32, tag="rbias0")
            nc.vector.memset(rbias0, LOG_SV6 - math.log(S_VT))
            rbias1 = pone.tile([D, 1], F32, tag="rbias1")
            nc.vector.memset(rbias1, LOG_SV6)
            texp = pone.tile([P, NPAIR], F32, tag="texp")
            nc.sync.dma_start(out=texp, in_=texp_d[:, :])

            _bcn = [0]

            def bcast(out_ap, row_ap, npart, width):
                """out[0:npart, :] = row broadcast across partitions (via a
                DRAM bounce; SBUF APs cannot have partition-step 0)."""
                _bcn[0] += 1
                drow = nc.dram_tensor(f"bcrow{_bcn[0]}", [1, width], row_ap.dtype)
                nc.sync.dma_start(out=drow[:, :], in_=row_ap[0:1, 0:width])
                src = drow[0:1, 0:width]
                bap = bass.AP(tensor=src.tensor, offset=src.offset,
                              ap=[[0, npart]] + list(src.ap[1:]))
                nc.sync.dma_start(out=out_ap[0:npart, 0:width], in_=bap)

            def bcast2(out_aps, row_tiles, width):
                """out_aps[i][0:64] = row_tiles[i] broadcast (base 0)."""
                _bcn[0] += 1
                drow = nc.dram_tensor(f"bcrow{_bcn[0]}", [2, width],
                                      row_tiles[0].dtype)
                for i in range(2):
                    nc.sync.dma_start(out=drow[i:i + 1, :],
                                      in_=row_tiles[i][0:1, 0:width])
                for i in range(2):
                    src = drow[i:i + 1, 0:width]
                    bap = bass.AP(tensor=src.tensor, offset=src.offset,
                                  ap=[[0, D]] + list(src.ap[1:]))
                    nc.sync.dma_start(out=out_aps[i][0:D, 0:width], in_=bap)

            if flags["qkb"]:
                qkb = pone.tile([P, 12], F32, tag="qkb")
                nc.sync.dma_start(out=qkb, in_=qkb_d[:, :])
            if flags["pjb"]:
                pjb = pone.tile([P, CO], F32, tag="pjb")
                nc.sync.dma_start(out=pjb, in_=pjb_d[:, :])
            if flags["f1b"]:
                f1b = pone.tile([P, HF], F32, tag="f1b")
                nc.sync.dma_start(out=f1b, in_=f1b_d[:, :])
            if flags["f2b"]:
                f2b = pone.tile([P, CO], F32, tag="f2b")
                nc.sync.dma_start(out=f2b, in_=f2b_d[:, :])
            if flags["ln1_aff"]:
                l1g = pone.tile([P, CO], F32, tag="l1g")
                nc.sync.dma_start(out=l1g, in_=l1g_d[:, :])
                l1b = pone.tile([P, CO], F32, tag="l1b")
                nc.sync.dma_start(out=l1b, in_=l1b_d[:, :])
            if flags["ln2_aff"]:
                l2g = pone.tile([P, CO], F32, tag="l2g")
                nc.sync.dma_start(out=l2g, in_=l2g_d[:, :])
                l2b = pone.tile([P, CO], F32, tag="l2b")
                nc.sync.dma_start(out=l2b, in_=l2b_d[:, :])
            if flags["vbias"]:
                vbrow = pone.tile([1, C], F32, tag="vbrow")
                nc.sync.dma_start(out=vbrow, in_=vbb_d[:, :])
                vbb = pone.tile([P, C], F32, tag="vbb")
                bcast(vbb, vbrow, P, C)
            if flags["mask"]:
                mqrow = pone.tile([1, TQ], F32, tag="mqrow")
                nc.sync.dma_start(out=mqrow, in_=mq_d[:, :])
                mqrow_b = pone.tile([1, TQ], BF16, tag="mqrow_b")
                nc.vector.tensor_copy(mqrow_b, mqrow)
                mqb = pone.tile([P, TQ], BF16, tag="mqb")
                bcast(mqb, mqrow_b, P, TQ)

            def rsqrt_row(var_row, tag="row"):
                """rs = exp(-0.5*ln(var+eps)); stays in the ln+exp set."""
                lnr = prow.tile([1, 512], F32, tag=tag)
                nc.scalar.activation(out=lnr, in_=var_row, func=AF.Ln,
                                     bias=eps_sb[0:1, :])
                rs_row = prow.tile([1, 512], F32, tag=tag)
                nc.scalar.activation(out=rs_row, in_=lnr, func=AF.Exp,
                                     scale=-0.5)
                return rs_row

            def layer_norm(src_of_co, aff, ht, use_act_sq=False):
                """Feature-major LN of one 512-token chunk into ht[:, co, :]."""
                mu_ps = psB.tile([1, 512], F32, tag="pb")
                m2_ps = psB.tile([1, 512], F32, tag="pb")
                srcs = []
                for co in range(CO):
                    s = src_of_co(co)
                    sq = psq.tile([P, 512], BF16, tag="sq")
                    if use_act_sq:
                        nc.scalar.activation(out=sq, in_=s, func=AF.Square)
                    else:
                        nc.vector.tensor_tensor(sq, s, s, OP.mult)
                    srcs.append(s)
                    nc.tensor.matmul(mu_ps, ones_b[:, :], s[:, :],
                                     start=(co == 0), stop=(co == CO - 1))
                    nc.tensor.matmul(m2_ps, ones_b[:, :], sq[:, :],
                                     start=(co == 0), stop=(co == CO - 1))
                mu_row = prow.tile([1, 512], BF16, tag="row")
                nc.vector.tensor_scalar_mul(mu_row, mu_ps, 1.0 / C)
                musq = prow.tile([1, 512], F32, tag="row")
                nc.vector.tensor_tensor(musq, mu_row, mu_row, OP.mult)
                var_row = prow.tile([1, 512], F32, tag="row")
                nc.vector.scalar_tensor_tensor(
                    out=var_row, in0=m2_ps, scalar=1.0 / C, in1=musq,
                    op0=OP.mult, op1=OP.subtract)
                rs_row = rsqrt_row(var_row)
                mu_b = pst.tile([P, 512], BF16, tag="st")
                bcast(mu_b, mu_row, P, 512)
                rs_b = pst.tile([P, 512], F32, tag="st")
                bcast(rs_b, rs_row, P, 512)

                def apply_ln():
                    # deferred so the DVE FIFO never blocks on the
                    # broadcast's DRAM round trip
                    for co in range(CO):
                        s = srcs[co]
                        u = pu.tile([P, 512], BF16, tag="u")
                        nc.vector.tensor_tensor(u, s, mu_b, OP.subtract)
                        if aff is not None:
                            g_, b_ = aff
                            nc.vector.tensor_tensor(u, u, rs_b, OP.mult)
                            nc.vector.tensor_scalar(
                                out=ht[:, co, :], in0=u,
                                scalar1=g_[:, co:co + 1],
                                scalar2=b_[:, co:co + 1],
                                op0=OP.mult, op1=OP.add)
                        else:
                            nc.vector.tensor_tensor(ht[:, co, :], u, rs_b,
                                                    OP.mult)
                return apply_ln

            # ---------------- LN1 (feature-major, per 512-token chunk) ------
            x6r = {}
            h1t = {}  # tchunk -> [128, CO, 512] f8
            ln_pend = []
            for t in range(NTC):
                x6 = px.tile([P, CO, 512], BF16, tag="x6", name=f"x6_{t}")
                nc.sync.dma_start(out=x6, in_=xTb3[:, :, t * 512:(t + 1) * 512])
                ht = ph1.tile([P, CO, 512], F8, tag="h1", name=f"h1t{t}")
                aff = (l1g, l1b) if flags["ln1_aff"] else None
                ln_pend.append(layer_norm(lambda co, x6=x6: x6[:, co, :],
                                          aff, ht, use_act_sq=True))
                if len(ln_pend) > 1:
                    ln_pend.pop(0)()
                h1t[t] = ht
                x6r[t] = x6
            while ln_pend:
                ln_pend.pop(0)()

            # ---------------- QKV projections (fp8 DoubleRow) ---------------
            kTs = [pkr.tile([P, TK], BF16, tag="kr", name=f"kTs{i}")
                   for i in range(NPAIR)]
            qTs = [pqr.tile([P, TQ], BF16, tag="qr", name=f"qTs{i}")
                   for i in range(NPAIR)]
            for f in range(12):
                is_q = f < 6
                ntp = 1 if is_q else 2   # q: only my 1024 tokens
                wt = pw.tile([P, CO, P], F8, tag="w")
                nc.sync.dma_start(out=wt, in_=wqk_d[f])
                for tp in range(ntp):
                    ps = psA.tile([P, 1024], F32, tag="pa")
                    for jp in range(3):
                        for th in range(2):
                            nc.tensor.matmul(
                                ps[:, th * 512:(th + 1) * 512],
                                wt[:, 2 * jp:2 * jp + 2, :],
                                h1t[tp * 2 + th][:, 2 * jp:2 * jp + 2, :],
                                start=(jp == 0), stop=(jp == 2), perf_mode=DR)
                    st = (qTs[f][:, :] if is_q
                          else kTs[f - 6][:, tp * 1024:(tp + 1) * 1024])
                    # drain on Act (idle during startup; DVE is the startup
                    # bottleneck). Copy is present in every table set.
                    if flags["qkb"]:
                        nc.scalar.activation(
                            out=st, in_=ps, func=AF.Copy,
                            scale=inv["qk"][f], bias=qkb[:, f:f + 1])
                    else:
                        nc.scalar.activation(out=st, in_=ps, func=AF.Copy,
                                             scale=inv["qk"][f])

            # v (ones col at 65-stride): even heads -> f8 pair tiles
            # (DR attn*V), odd heads -> bf16 pair tiles
            wv_sb = pwb.tile([P, CO, C], F8, tag="wbig")
            nc.sync.dma_start(out=wv_sb, in_=wv_d[:, :, :])
            vtf = {}
            vtb = {}
            for gp in range(8):
                vf = pvf.tile([P, 2, VP2], F8, tag="vf", name=f"vf{gp}")
                vb = pvb.tile([P, 2, VP2], BF16, tag="vb", name=f"vb{gp}")
                vf4 = vf[:, :, 0:390].rearrange("p k (h e) -> p k h e", e=65)
                vb4 = vb[:, :, 0:390].rearrange("p k (h e) -> p k h e", e=65)
                nc.vector.memset(vf4[:, :, :, D:65], 1.0)
                nc.vector.memset(vb4[:, :, :, D:65], 1.0)
                vtf[gp] = (vf, vf4)
                vtb[gp] = (vb, vb4)
            for ts_ in range(NKT):
                gp, ktl = ts_ // 2, ts_ % 2
                lt = ts_ // 4
                sub = ts_ % 4
                ps = psA.tile([P, 1024], F32, tag="pa")
                for jp in range(3):
                    lhs = h1t[lt][:, 2 * jp:2 * jp + 2, sub * P:(sub + 1) * P]
                    nc.tensor.matmul(ps[:, 0:512], lhs,
                                     wv_sb[:, 2 * jp:2 * jp + 2, 0:512],
                                     start=(jp == 0), stop=(jp == 2),
                                     perf_mode=DR)
                    nc.tensor.matmul(ps[:, 512:768], lhs,
                                     wv_sb[:, 2 * jp:2 * jp + 2, 512:768],
                                     start=(jp == 0), stop=(jp == 2),
                                     perf_mode=DR)
                if flags["vbias"]:
                    nc.vector.tensor_tensor(ps[:, 0:768], ps[:, 0:768],
                                            vbb[:, :], OP.add)
                ph4 = ps[:, 0:768].rearrange("p (h two e) -> p h two e",
                                             two=2, e=D)
                nc.scalar.activation(out=vtf[gp][1][:, ktl, :, 0:D],
                                     in_=ph4[:, :, 0, :], func=AF.Copy,
                                     scale=inv["v"] * S_VT)
                nc.scalar.activation(out=vtb[gp][1][:, ktl, :, 0:D],
                                     in_=ph4[:, :, 1, :], func=AF.Copy,
                                     scale=inv["v"])

            # ---------------- attention (both query chunks) -----------------
            val6s = {}
            for qc in range(NQC):
                qsl = slice(qc * 512, (qc + 1) * 512)
                val6 = pvl.tile([P, CO, 512], F8, tag="vl", name=f"vl{qc}")
                val6s[qc] = val6
                stageB = []
                stageC = []

                def emit_stageB(p_, rbt_, pvd_):
                    rb_ = [prc.tile([D, 512], BF16, tag="rb",
                                    name=f"rb{qc}_{p_}_{s_}")
                           for s_ in range(2)]
                    for s_ in range(2):
                        nc.scalar.activation(out=rb_[s_], in_=rbt_[s_],
                                             func=AF.Exp, scale=-1.0,
                                             bias=(rbias0 if s_ == 0
                                                   else rbias1)[:, :])
                    stageC.append((p_, rb_, pvd_))

                def emit_stageC(p_, rb_, pvd_):
                    for s_ in range(2):
                        nc.vector.tensor_tensor(
                            val6[s_ * D:(s_ + 1) * D, p_, :],
                            pvd_[s_], rb_[s_], OP.mult)

                for p in range(NPAIR):
                    kT = kTs[p]
                    qT = qTs[p]
                    pv_ps = [psB.tile([65, 512], F32, tag="pb",
                                      name=f"pv{qc}_{p}_{s_}")
                             for s_ in range(2)]
                    ats = {}

                    def attn_v(g):
                        # s=0 (even head): one fp8-DR matmul contracting both
                        # key tiles of the group
                        nc.tensor.matmul(
                            pv_ps[0][:, :],
                            vtf[g][0][:, :, p * 65:p * 65 + 65],
                            ats[g][0].rearrange("p (k q) -> p k q", k=2),
                            start=(g == 0), stop=(g == 7), perf_mode=DR)
                        # s=1 (odd head): bf16
                        for ktl in range(2):
                            nc.tensor.matmul(
                                pv_ps[1][:, :],
                                vtb[g][0][:, ktl, p * 65:p * 65 + 65],
                                ats[g][1][:, ktl * 512:(ktl + 1) * 512],
                                start=(g == 0 and ktl == 0),
                                stop=(g == 7 and ktl == 1))

                    # depth-2 software pipeline: attn*V for group g-1 issues
                    # after the exps of group g, so each exp has a full PE
                    # cycle to complete (no strict-FIFO stall) and both
                    # heads' score matmuls are ready together (adjacent
                    # h0/h64 row groups run concurrently in the array).
                    for g in range(8):
                        if g > 0:
                            attn_v(g - 1)
                        sc = [psA.tile([P, 1024], F32, tag="pa",
                                       name=f"sc{qc}_{p}_{g}_{s_}")
                              for s_ in range(2)]
                        for ktl in range(2):
                            kt = g * 2 + ktl
                            for s in range(2):
                                nc.tensor.matmul(
                                    sc[s][:, ktl * 512:(ktl + 1) * 512],
                                    kT[s * D:(s + 1) * D, kt * P:(kt + 1) * P],
                                    qT[s * D:(s + 1) * D, qsl],
                                    start=True, stop=True)
                        if flags["mask"]:
                            for s in range(2):
                                for ktl in range(2):
                                    nc.vector.tensor_tensor(
                                        sc[s][:, ktl * 512:(ktl + 1) * 512],
                                        sc[s][:, ktl * 512:(ktl + 1) * 512],
                                        mqb[:, qsl], OP.mult)
                        # exp: s=0 on Act (exact), s=1 on DVE (Schraudolph)
                        at0 = pat.tile([P, 1024], F8, tag="at",
                                       name=f"at{qc}_{p}_{g}_0")
                        at1 = patb.tile([P, 1024], BF16, tag="at",
                                       name=f"at{qc}_{p}_{g}_1")
                        nc.scalar.activation(out=at0, in_=sc[0][:, :],
                                             func=AF.Exp,
                                             bias=texp[:, p:p + 1])
                        nc.vector.tensor_scalar(
                            out=at1[:, :].bitcast(I16), in0=sc[1][:, :],
                            scalar1=EXP_A, scalar2=EXP_B,
                            op0=OP.mult, op1=OP.add)
                        ats[g] = (at0, at1)
                    attn_v(7)
                    # per-pair tail, software-pipelined so no engine FIFO
                    # ever waits on the rb broadcast's DRAM round trip:
                    #   stage A (now): drain raw sums to SBUF + ln rows +
                    #     launch broadcast DMAs
                    #   stage B (next pair): rb = exp(-lnz + bias)
                    #   stage C (pair after): val6 = pvd * rb (f8 quantize)
                    pvd = [ppd.tile([D, 512], BF16, tag="pd",
                                    name=f"pvd{qc}_{p}_{s_}")
                           for s_ in range(2)]
                    for s in range(2):
                        nc.vector.tensor_copy(pvd[s], pv_ps[s][0:D, :])
                    lnz = [prow.tile([1, 512], F32, tag="row",
                                     name=f"lnz{qc}_{p}_{s_}")
                           for s_ in range(2)]
                    for s in range(2):
                        nc.scalar.activation(out=lnz[s], in_=pv_ps[s][64:65, :],
                                             func=AF.Ln)
                    rbt = [prb.tile([D, 512], F32, tag="rbt",
                                    name=f"rbt{qc}_{p}_{s_}")
                           for s_ in range(2)]
                    bcast2(rbt, lnz, 512)
                    stageB.append((p, rbt, pvd))
                    if len(stageB) > 1:
                        emit_stageB(*stageB.pop(0))
                    if len(stageC) > 1:
                        emit_stageC(*stageC.pop(0))

                while stageB:
                    emit_stageB(*stageB.pop(0))
                while stageC:
                    emit_stageC(*stageC.pop(0))

            # ---------------- tails: proj+LN2+fc1+fc2 (both chunks) ---------
            x2s = {}
            xqs = {}
            for qc in range(NQC):
                xq = px.tile([P, CO, 512], BF16, tag="x6", name=f"xq_{qc}")
                nc.sync.dma_start(
                    out=xq, in_=xTb3[:, :, qc * 512:(qc + 1) * 512])
                xqs[qc] = xq
            for qc in range(NQC):
                val6 = val6s[qc]
                x2 = px2.tile([P, CO, 512], BF16, tag="x2", name=f"x2_{qc}")
                x2s[qc] = x2
                for of in range(CO):
                    wt = pw.tile([P, CO, P], F8, tag="w", name=f"pj{qc}_{of}")
                    nc.sync.dma_start(out=wt, in_=pjw_d[of])
                    ps = psA.tile([P, 512], F32, tag="pa")
                    for jp in range(3):
                        nc.tensor.matmul(
                            ps[:, :], wt[:, 2 * jp:2 * jp + 2, :],
                            val6[:, 2 * jp:2 * jp + 2, :],
                            start=(jp == 0), stop=(jp == 2), perf_mode=DR)
                    if flags["pjb"]:
                        a2t = pu.tile([P, 512], F32, tag="u")
                        nc.vector.tensor_scalar(
                            out=a2t, in0=ps, scalar1=inv["pj"],
                            scalar2=pjb[:, of:of + 1], op0=OP.mult, op1=OP.add)
                        nc.vector.tensor_tensor(
                            x2[:, of, :], a2t, xqs[qc][:, of, :], OP.add)
                    else:
                        # x2 = proj*inv + xb in one pass
                        nc.vector.scalar_tensor_tensor(
                            out=x2[:, of, :], in0=ps, scalar=inv["pj"],
                            in1=xqs[qc][:, of, :], op0=OP.mult, op1=OP.add)

            h2s = {}
            ln2_pend = []
            for qc in range(NQC):
                h2 = ph2.tile([P, CO, 512], BF16, tag="h2", name=f"h2t{qc}")
                aff2 = (l2g, l2b) if flags["ln2_aff"] else None
                ln2_pend.append(layer_norm(
                    lambda co, qc=qc: x2s[qc][:, co, :], aff2, h2,
                    use_act_sq=False))
                h2s[qc] = h2
            while ln2_pend:
                ln2_pend.pop(0)()

            hids = {}
            for qc in range(NQC):
                hid = phid.tile([P, HF, 512], F8, tag="hid", name=f"hid{qc}")
                hids[qc] = hid
                h2 = h2s[qc]
                for hf in range(HF):
                    wt = pw.tile([P, CO, P], BF16, tag="w",
                                 name=f"w1_{qc}_{hf}")
                    nc.sync.dma_start(out=wt, in_=f1w_d[hf])
                    ps = psA.tile([P, 512], F32, tag="pa",
                                  name=f"f1p{qc}_{hf}")
                    for co in range(CO):
                        nc.tensor.matmul(
                            ps, wt[:, co, :], h2[:, co, :],
                            start=(co == 0), stop=(co == CO - 1))
                    bias = f1b[:, hf:hf + 1] if flags["f1b"] else 0.0
                    nc.scalar.activation(out=hid[:, hf, :], in_=ps,
                                         func=AF.Gelu, bias=bias)

            for qc in range(NQC):
                qsl = slice(qc * 512, (qc + 1) * 512)
                hid = hids[qc]
                for of in range(CO):
                    wt2 = pwb.tile([P, HF, P], F8, tag="wbig",
                                   name=f"w2_{qc}_{of}")
                    nc.sync.dma_start(out=wt2, in_=f2w_d[of])
                    ps = psA.tile([P, 512], F32, tag="pa",
                                  name=f"f2p{qc}_{of}")
                    for hp in range(HF // 2):
                        nc.tensor.matmul(
                            ps, wt2[:, 2 * hp:2 * hp + 2, :],
                            hid[:, 2 * hp:2 * hp + 2, :],
                            start=(hp == 0), stop=(hp == HF // 2 - 1),
                            perf_mode=DR)
                    ot = pxy.tile([P, 512], BF16, tag="xmy",
                                  name=f"ot{qc}_{of}")
                    if flags["f2b"]:
                        nc.vector.tensor_scalar(
                            out=ot[:, :], in0=ps, scalar1=inv["f2"],
                            scalar2=f2b[:, of:of + 1], op0=OP.mult, op1=OP.add)
                        nc.vector.tensor_tensor(ot, ot, x2s[qc][:, of, :],
                                                OP.add)
                    else:
                        # out = fc2*inv + x2  (x2 already includes bf16(x))
                        nc.vector.scalar_tensor_tensor(
                            out=ot[:, :], in0=ps, scalar=inv["f2"],
                            in1=x2s[qc][:, of, :], op0=OP.mult, op1=OP.add)
                    nc.sync.dma_start(
                        out=out_d[of * P:(of + 1) * P, qsl],
                        in_=ot[:, :])

    nc.compile()
    return nc


_CACHE = {}
RUN_KWARGS = {}     # test harness can set {"trace": True}
LAST_RESULT = None  # BassKernelResults of the last kernel() call


def _f32(a):
    return np.ascontiguousarray(np.asarray(a, dtype=np.float32))


def _f8(a):
    return np.ascontiguousarray(
        np.clip(np.asarray(a, np.float32), -448.0, 448.0).astype(
            ml_dtypes.float8_e4m3fn))


def _pow2_scale(absmax):
    """Power-of-two scale putting absmax around 224 (half of e4m3 max)."""
    absmax = float(absmax)
    if absmax <= 0 or not math.isfinite(absmax):
        return 1.0
    return 2.0 ** math.floor(math.log2(224.0 / absmax))


def kernel(x, mask, ln1_g, ln1_b, qkv_w, qkv_b, proj_w, proj_b,
           ln2_g, ln2_b, fc1_w, fc1_b, fc2_w, fc2_b):
    x = _f32(x); mask = np.asarray(mask)
    ln1_g = _f32(ln1_g); ln1_b = _f32(ln1_b)
    qkv_w = _f32(qkv_w); qkv_b = _f32(qkv_b)
    proj_w = _f32(proj_w); proj_b = _f32(proj_b)
    ln2_g = _f32(ln2_g); ln2_b = _f32(ln2_b)
    fc1_w = _f32(fc1_w); fc1_b = _f32(fc1_b)
    fc2_w = _f32(fc2_w); fc2_b = _f32(fc2_b)
    B, N, Cx = x.shape
    assert (B, N, Cx) == (4, 2048, 768)

    scale = D ** -0.5
    qkv_ws = qkv_w.copy()
    qkv_ws[:, :C] *= scale
    qkv_bs = qkv_b.copy()
    qkv_bs[:C] *= scale

    flags = {
        "ln1_aff": not (np.all(ln1_g == 1) and np.all(ln1_b == 0)),
        "ln2_aff": not (np.all(ln2_g == 1) and np.all(ln2_b == 0)),
        "vbias": not np.all(qkv_bs[2 * C:] == 0),
        "mask": not np.all(mask == 1),
        "qkb": not np.all(qkv_bs[:2 * C] == 0),
        "pjb": not np.all(proj_b == 0),
        "f1b": not np.all(fc1_b == 0),
        "f2b": not np.all(fc2_b == 0),
    }

    def tile_lhs(w, nf):
        # w [K, nf*128] -> [nf, 128(ci), K//128(co), 128] contiguous
        K = w.shape[0]
        co = K // P
        r = w.reshape(co, P, nf, P)            # [co, ci, f, j]
        return np.ascontiguousarray(r.transpose(2, 1, 0, 3))  # [f, ci, co, j]

    wqk_t = tile_lhs(qkv_ws[:, :2 * C], 12)
    s_qk = [_pow2_scale(np.max(np.abs(wqk_t[f]))) for f in range(12)]
    wqk = _f8(wqk_t * np.asarray(s_qk, np.float32)[:, None, None, None])
    wv_t = qkv_ws[:, 2 * C:].reshape(CO, P, C).transpose(1, 0, 2)
    s_v = _pow2_scale(np.max(np.abs(wv_t)))
    wv = _f8(wv_t * s_v)
    pjw_t = tile_lhs(proj_w, CO)
    s_pj = _pow2_scale(np.max(np.abs(pjw_t)))
    pjw = _f8(pjw_t * s_pj)
    f1w = np.ascontiguousarray(tile_lhs(fc1_w, HF).astype(ml_dtypes.bfloat16))
    f2w_t = tile_lhs(fc2_w, CO)
    s_f2 = _pow2_scale(np.max(np.abs(f2w_t)))
    f2w = _f8(f2w_t * s_f2)

    inv = {
        "qk": [1.0 / s for s in s_qk],
        "v": 1.0 / s_v,
        "pj": 1.0 / (s_pj * S_V6),
        "f2": 1.0 / s_f2,
    }

    # sample-based per-even-head score shift T so Act exp output fits f8:
    # at0 = exp(s - T_p) with T_p = sampled max + margin (saturation at 448
    # degrades gracefully if the true max exceeds the sample max by > margin)
    T = np.zeros(NPAIR, np.float32)
    step = max(1, N // 256)
    for b in range(B):
        xs = x[b, ::step, :].astype(np.float32)
        mu = xs.mean(-1, keepdims=True)
        var = ((xs - mu) ** 2).mean(-1, keepdims=True)
        hs = (xs - mu) / np.sqrt(var + LN_EPS) * ln1_g + ln1_b
        qs = (hs @ qkv_ws[:, :C] + qkv_bs[:C]).reshape(-1, H, D)
        ks = (hs @ qkv_ws[:, C:2 * C] + qkv_bs[C:2 * C]).reshape(-1, H, D)
        if flags["mask"]:
            mb = mask[b, ::step].astype(np.float32)
        for p_ in range(NPAIR):
            sc_ = np.einsum("qd,kd->qk", qs[:, 2 * p_], ks[:, 2 * p_])
            if flags["mask"]:
                sc_ = sc_ * mb[:, None]
            T[p_] = max(T[p_], sc_.max())
    T += 2.0

    key = (tuple(sorted(flags.items())),
           tuple(s_qk), s_v, s_pj, s_f2)
    if key not in _CACHE:
        _CACHE[key] = _build_nc(flags, inv)
    nc = _CACHE[key]

    shared = {"wqk": wqk, "wv": wv, "pjw": pjw, "f1w": f1w, "f2w": f2w,
              "texp": np.ascontiguousarray(
                  np.tile(-T[None, :], (P, 1)).astype(np.float32))}
    if flags["qkb"]:
        shared["qkb"] = np.ascontiguousarray(qkv_bs[:2 * C].reshape(12, P).T)
    if flags["pjb"]:
        shared["pjb"] = np.ascontiguousarray(proj_b.reshape(CO, P).T)
    if flags["f1b"]:
        shared["f1b"] = np.ascontiguousarray(fc1_b.reshape(HF, P).T)
    if flags["f2b"]:
        shared["f2b"] = np.ascontiguousarray(fc2_b.reshape(CO, P).T)
    if flags["ln1_aff"]:
        shared["l1g"] = np.ascontiguousarray(ln1_g.reshape(CO, P).T)
        shared["l1b"] = np.ascontiguousarray(ln1_b.reshape(CO, P).T)
    if flags["ln2_aff"]:
        shared["l2g"] = np.ascontiguousarray(ln2_g.reshape(CO, P).T)
        shared["l2b"] = np.ascontiguousarray(ln2_b.reshape(CO, P).T)
    if flags["vbias"]:
        shared["vbb"] = np.ascontiguousarray(
            (qkv_bs[2 * C:] * s_v).reshape(1, C))

    in_maps = []
    xbf = x.astype(ml_dtypes.bfloat16)
    for c in range(8):
        b, half = c // 2, c % 2
        xb = xbf[b]
        xr = np.concatenate([xb[half * TQ:(half + 1) * TQ],
                             xb[(1 - half) * TQ:(2 - half) * TQ]], axis=0)
        m = dict(shared)
        m["xTb"] = np.ascontiguousarray(xr.T)
        if flags["mask"]:
            mr = mask[b].astype(np.float32)[half * TQ:(half + 1) * TQ]
            m["mq"] = np.ascontiguousarray(mr.reshape(1, TQ))
        in_maps.append(m)

    res = run_bass_kernel_spmd(nc, in_maps, core_ids=list(range(8)), **RUN_KWARGS)
    global LAST_RESULT
    LAST_RESULT = res
    # device returns x2 + mlp with x2 = bf16(x) + attn_proj; add back the
    # f32-vs-bf16 residue of x on the host.
    xres = x - xbf.astype(np.float32)
    out = np.empty((B, N, C), np.float32)
    for c in range(8):
        b, half = c // 2, c % 2
        sl = slice(half * TQ, (half + 1) * TQ)
        out[b, sl, :] = res.results[c]["outT"].T.astype(np.float32) + xres[b, sl]
    return out
